# revision 5
# baseline (speedup 1.0000x reference)
"""PoissonGaussianReadout forward on 8 trn2 NeuronCores.

Math (eval mode): each neuron n samples feat[b] (a [36,36,1024] image per
batch, 1024 = C*T channels) bilinearly at a fixed point mu[n], then takes a
per-neuron dot with W[n,:], adds b[n], applies elu(y)+1.

Strategy:
  - Hybrid shard 4x2: 8 cores = 4 batch-groups (4 batches each) x 2 halves
    of the contraction dim D (512 channels each).  Cores emit LINEAR
    partial sums; the host adds the halves, bias, and elu on [16,4096].
  - fp8(e4m3) x and W with DoubleRow matmuls; per-neuron dequant folded
    into a bf16 mask.
  - Sort neurons by bilinear base cell p00; 32 blocks of 128 neurons,
    each spanning a window of <=93 flat positions.  Two DoubleRow
    matmuls per block: psum[n, (b,j)] += Wblk^T @ feat-window.
  - The bilinear mask-reduce is split across engine pipelines so it is no
    longer a single-engine critical path:
      P0 blocks: DVE scalar_tensor_tensor straight from PSUM (one per
        batch, fused mult+accum).
      P2 blocks: Act copies psum -> SBUF fp32 (frees the psum bank
        early), GpSimd multiplies by the bf16 mask (stride-0 broadcast
        over the 4 batches), DVE tensor_reduce(axis=X) emits all 4 z
        values in one op.
    DVE and GpSimd run concurrently; the mix ratio balances them.
  - DMA: feat cut into 4 block-group segments; every large transfer is
    issued as TWO half-DMAs back-to-back because a single HWDGE queue
    entry only sustains ~0.2 MB/us while two concurrent entries reach
    ~0.4 MB/us.  Everything need-ordered and ungated on the sync queue:
    [W0, M0, F0, W1, F1, W2, F2, W3, F3]; the remaining mask groups ride
    the (slow-starting) scalar-engine queue; z stores issued last.
"""
import sys
sys.path.insert(0, "/opt/trn_rl_repo")

import numpy as np

from concourse import bass, mybir, tile
from concourse.bass_utils import run_bass_kernel_spmd
import bass_rust

# problem constants
B, C, T, HH, WW = 16, 64, 16, 36, 36
N, D = 4096, C * T             # 4096 neurons, 1024 input dim
P = HH * WW                    # 1296 flat positions
NCORES = 8
NBG = 4                        # batch groups
NDH = 2                        # D halves
BPC = B // NBG                 # batches per core = 4
DH = D // NDH                  # channels per core = 512
NC2 = DH // 256                # 2 double-subtile (256-chan) passes per core
PAD = 38                       # max corner offset (37) + 1
WINMAX = 128                   # psum bank: BPC*WIN <= 512 fp32
GSIZES = (4, 7, 7, 7, 7)       # ragged block groups: small first group so
                               # the reduce pipeline starts ~1.5us earlier

F32 = mybir.dt.float32
BF16 = mybir.dt.bfloat16

import ml_dtypes
F8_DT = mybir.dt.float8e4
F8_NP = ml_dtypes.float8_e4m3   # max normal 240
F8_CAP = np.float32(224.0)


def _is_p3(i, nblk):
    """Blocks offloaded to the DVE-free pipeline (Act copy -> GpSimd
    mask-mult -> Act activation+accum x4).  DVE keeps pure stt chains for
    the rest (mixing op types on DVE costs ~60ns/op).  Balance: DVE block
    ~664ns vs Act ~1540ns -> ~10 of 32 blocks offloaded."""
    if i >= nblk - 2:
        return False            # lowest-latency path for the final blocks
    return (i % 3) == 1


def _split_waits(nc, max_waits=1):
    """Walrus in this image allows only ONE sem wait per instruction.
    Hoist extra waits onto injected same-engine NoOps placed immediately
    before the owning instruction (same engine + program order => same
    semantics)."""
    k = 0
    for fn in nc.m.functions:
        for blk in fn.blocks:
            insts = blk.instructions
            out = []
            for inst in insts:
                si = inst.sync_info
                if si is not None and si.on_wait and len(si.on_wait) > max_waits:
                    waits = list(si.on_wait)
                    for w in waits[:-max_waits]:
                        nop = mybir.InstNoOp(name=f"I-wsplit-{k}", ins=[], outs=[])
                        k += 1
                        nop.engine = inst.engine
                        nop.sync_info = bass_rust.SyncInfo(
                            on_wait=[w], on_update=[]
                        )
                        out.append(nop)
                    si.on_wait = waits[-max_waits:]
                    inst.sync_info = si
                out.append(inst)
            if len(out) != len(insts):
                insts.clear()
                insts.extend(out)


def _bilinear_tables(mu):
    """Per-neuron base cell p00, corner offsets (4) in {0,1,36,37}, corner
    weights (4), replicating reference float32 arithmetic exactly."""
    one, half = np.float32(1.0), np.float32(0.5)
    g = np.clip(mu.astype(np.float32), -one, one)
    ix = (g[:, 0] + one) * np.float32(WW * 0.5) - half
    iy = (g[:, 1] + one) * np.float32(HH * 0.5) - half
    x0 = np.floor(ix)
    y0 = np.floor(iy)
    wx1 = ix - x0
    wx0 = one - wx1
    wy1 = iy - y0
    wy0 = one - wy1

    xs = [x0, x0 + one]
    ys = [y0, y0 + one]
    wxs = [wx0, wx1]
    wys = [wy0, wy1]

    x0c = np.clip(x0, 0, WW - 1).astype(np.int64)
    y0c = np.clip(y0, 0, HH - 1).astype(np.int64)
    p00 = y0c * WW + x0c

    offs = np.zeros((4, N), np.int64)
    wgts = np.zeros((4, N), np.float32)
    k = 0
    for a in range(2):          # y corner
        for bb in range(2):     # x corner
            xx, yy = xs[bb], ys[a]
            valid = (xx >= 0) & (xx <= WW - 1) & (yy >= 0) & (yy <= HH - 1)
            xi = np.clip(xx, 0, WW - 1).astype(np.int64)
            yi = np.clip(yy, 0, HH - 1).astype(np.int64)
            offs[k] = yi * WW + xi - p00
            wgts[k] = (wys[a] * wxs[bb]) * valid.astype(np.float32)
            k += 1
    assert offs.min() >= 0 and offs.max() <= 37
    return p00, offs, wgts


def _make_blocks(p00_sorted):
    """Greedy blocks of <=128 sorted neurons with window <= WINMAX."""
    blocks = []  # (start, end) into sorted order
    s = 0
    n = len(p00_sorted)
    while s < n:
        pfirst = p00_sorted[s]
        e = s
        while e < n and e - s < 128 and (p00_sorted[e] - pfirst) + PAD <= WINMAX:
            e += 1
        blocks.append((s, e))
        s = e
    return blocks


def kernel(x, mu, sigma, W, b):
    x = np.ascontiguousarray(x, dtype=np.float32)
    W = np.ascontiguousarray(W, dtype=np.float32)
    b = np.asarray(b, dtype=np.float32)

    p00, offs, wgts = _bilinear_tables(mu)
    order = np.argsort(p00, kind="stable")
    p00s = p00[order]
    blocks = _make_blocks(p00s)
    nblk = len(blocks)
    gbounds = [0]
    for gs in GSIZES:
        gbounds.append(min(gbounds[-1] + gs, nblk))
    while gbounds[-1] < nblk:
        gbounds.append(min(gbounds[-1] + GSIZES[-1], nblk))
    gbounds = sorted(set(gbounds))
    ngrp = len(gbounds) - 1

    # ---- fp8 quantization: global x scale, per-neuron W scale ----
    sx = F8_CAP / np.float32(max(np.abs(x).max(), 1e-30))
    sw = F8_CAP / np.maximum(np.abs(W).max(axis=1), 1e-30).astype(np.float32)
    Wq = (W * sw[:, None]).astype(F8_NP)    # [N, D]
    dequant = 1.0 / (sw * sx)               # [N] folded into the mask

    # per-block host data
    wins, pfirsts, ms, sblk = [], [], [], []
    mparts = []
    for i, (s, e) in enumerate(blocks):
        idx = order[s:e]
        m = e - s
        pfirst = int(p00s[s])
        win = int(p00s[e - 1]) - pfirst + PAD
        ms.append(m)
        pfirsts.append(pfirst)
        wins.append(win)
        sblk.append(s)
        # mask [128, win], fp8 dequant folded in
        mk = np.zeros((128, win), np.float32)
        rel = (p00[idx] - pfirst)  # [m]
        for k in range(4):
            np.add.at(mk[:m], (np.arange(m), rel + offs[k][idx]),
                      wgts[k][idx] * dequant[idx])
        mparts.append(mk)
    sblk.append(N)

    mask_all = np.ascontiguousarray(
        np.concatenate(mparts, axis=1)).astype(ml_dtypes.bfloat16)
    moffs = np.cumsum([0] + [w for w in wins])
    mtot = int(mask_all.shape[1])

    # feat segments: one per block group; window-union of its blocks
    seg_lo, seg_w = [], []
    for g in range(ngrp):
        lo = pfirsts[gbounds[g]]
        hi = max(pfirsts[i] + wins[i] for i in range(gbounds[g], gbounds[g + 1]))
        seg_lo.append(lo)
        seg_w.append(hi - lo)

    # W packed per group with one contiguous row per partition:
    # group layout [128, NC2, 2, sum_m(group)].
    Ws = Wq[order]                          # [N, D] sorted
    gms = [sblk[gbounds[g + 1]] - sblk[gbounds[g]] for g in range(ngrp)]
    wgrps = []                              # [dh][g] -> [128, NC2, 2, gm]
    for dh in range(NDH):
        wl = (Ws[:, dh * DH:(dh + 1) * DH].T        # [512, N]
              .reshape(NC2, 2, 128, N).transpose(2, 0, 1, 3))  # [128,NC2,2,N]
        parts = []
        for g in range(ngrp):
            lo, hi = sblk[gbounds[g]], sblk[gbounds[g + 1]]
            parts.append(np.ascontiguousarray(wl[:, :, :, lo:hi]))
        wgrps.append(parts)

    # ---- build the Bass program (same for all cores) ----
    nc = bass.Bass()
    xs_hs = [nc.declare_dram_parameter(f"xs{s}", [128, NC2, 2, BPC, seg_w[s]],
                                       F8_DT, isOutput=False)
             for s in range(ngrp)]
    wf_hs = [nc.declare_dram_parameter(f"wf{g}", [128, NC2, 2, gms[g]],
                                       F8_DT, isOutput=False)
             for g in range(ngrp)]
    mf_h = nc.declare_dram_parameter("mf", [128, mtot], BF16, isOutput=False)
    z_h = nc.declare_dram_parameter("z", [128, BPC * nblk], F32, isOutput=True)

    ADD = mybir.AluOpType.add
    MULT = mybir.AluOpType.mult
    AXX = mybir.AxisListType.X
    DR = mybir.MatmulPerfMode.DoubleRow

    with tile.TileContext(nc) as tc:
        with (
            tc.tile_pool(name="feat", bufs=1) as featp,
            tc.tile_pool(name="wpool", bufs=1) as wpool,
            tc.tile_pool(name="mpool", bufs=1) as mpool,
            tc.tile_pool(name="spool", bufs=4) as spool,
            tc.tile_pool(name="stg", bufs=4) as stgp,
            tc.tile_pool(name="mk", bufs=4) as mkp,
            tc.tile_pool(name="zpool", bufs=1) as zpool,
            tc.tile_pool(name="psum", bufs=1, space="PSUM") as psump,
        ):
            fts = [featp.tile([128, NC2, 2, BPC, seg_w[s]], F8_DT,
                              name=f"feat{s}") for s in range(ngrp)]
            mask_t = mpool.tile([128, mtot], BF16)
            zAll = zpool.tile([128, BPC * nblk], F32)
            wgs = [wpool.tile([128, NC2, 2, gms[g]], F8_DT, name=f"wg{g}")
                   for g in range(ngrp)]

            # need-ordered, ungated DMA stream on the sync queue; feat (and
            # the first W group) split in NC2 halves so TWO queue entries
            # stay in flight (a single entry only sustains ~0.2 MB/us).
            for c in range(NC2):
                nc.sync.dma_start(wgs[0][:, c], wf_hs[0][:, c])
            for c in range(NC2):
                nc.sync.dma_start(fts[0][:, c], xs_hs[0][:, c])
            for g in range(1, ngrp):
                nc.sync.dma_start(wgs[g][:], wf_hs[g][:])
                for c in range(NC2):
                    nc.sync.dma_start(fts[g][:, c], xs_hs[g][:, c])
            # all masks ride the scalar queue, grouped by first need time
            mcuts = [0, int(moffs[gbounds[min(2, ngrp)]]),
                     int(moffs[gbounds[min(4, ngrp)]]), mtot]
            mcuts = sorted(set(mcuts))
            for a, bnd in zip(mcuts[:-1], mcuts[1:]):
                nc.scalar.dma_start(mask_t[:, a:bnd], mf_h[:, a:bnd])

            for g in range(ngrp):
                blks = list(range(gbounds[g], gbounds[g + 1]))
                wg = wgs[g]
                glo = sblk[gbounds[g]]
                for i in blks:
                    m, win, pfirst = ms[i], wins[i], pfirsts[i]
                    o = sblk[i] - glo
                    off = pfirst - seg_lo[g]
                    pm = psump.tile([128, BPC, win], F32,
                                    name=f"pm{i}", tag=f"pm{i % 8}")
                    for c in range(NC2):
                        nc.tensor.matmul(
                            pm[0:m, :, :],
                            wg[:, c, :, o:o + m],
                            fts[g][:, c, :, :, off:off + win],
                            start=(c == 0),
                            stop=(c == NC2 - 1),
                            perf_mode=DR,
                        )
                    mo = int(moffs[i])
                    if not _is_p3(i, nblk):
                        # DVE: fused mask-mult + accum straight out of PSUM
                        for bb in range(BPC):
                            sc = spool.tile([128, WINMAX], F32, tag=f"sv{bb}")
                            nc.vector.scalar_tensor_tensor(
                                sc[0:m, 0:win],
                                pm[0:m, bb, :],
                                0.0,
                                mask_t[0:m, mo:mo + win],
                                ADD,
                                MULT,
                                accum_out=zAll[0:m,
                                               BPC * i + bb:BPC * i + bb + 1],
                            )
                    else:
                        # Act: psum -> SBUF fp32 (frees the bank early)
                        stg = stgp.tile([128, BPC, WINMAX], F32,
                                        tag=f"st{i % 4}")
                        nc.scalar.copy(stg[0:m, :, 0:win], pm[0:m, :, :])
                        # GpSimd: multiply by mask broadcast over batches
                        base = mask_t[0:m, mo:mo + win]
                        bap = [list(p) for p in base.ap]
                        bc = bass.AP(base.tensor, base.offset,
                                     [bap[0], [0, BPC], bap[1]])
                        mk = mkp.tile([128, BPC, WINMAX], F32,
                                      tag=f"mk{i % 4}")
                        nc.gpsimd.tensor_tensor(
                            mk[0:m, :, 0:win], stg[0:m, :, 0:win], bc, MULT)
                        # Act: per-batch accumulate (keeps DVE stt-only)
                        for bb in range(BPC):
                            sa = spool.tile([128, WINMAX], F32, tag=f"sa{bb}")
                            nc.scalar.activation(
                                sa[0:m, 0:win],
                                mk[0:m, bb, 0:win],
                                mybir.ActivationFunctionType.Copy,
                                accum_out=zAll[0:m,
                                               BPC * i + bb:BPC * i + bb + 1],
                            )
            # z stores: issued after all input triggers; sync idle by then
            zcuts = sorted(set((0, gbounds[min(3, ngrp)], nblk - 4, nblk)))
            for a, bnd in zip(zcuts[:-1], zcuts[1:]):
                sl = slice(BPC * a, BPC * bnd)
                nc.sync.dma_start(z_h[:, sl], zAll[:, sl])

    _split_waits(nc)

    # ---- run on 8 cores: core id = bg*2 + dh ----
    xq = (x.reshape(B, D // 128, 128, P) * sx).astype(F8_NP)
    in_maps = []
    for core in range(NCORES):
        bg, dh = core // NDH, core % NDH
        blkx = xq[BPC * bg:BPC * (bg + 1),
                  4 * dh:4 * (dh + 1)].reshape(BPC, NC2, 2, 128, P)
        im = {"mf": mask_all}
        for g in range(ngrp):
            lo, w_ = seg_lo[g], seg_w[g]
            seg = np.zeros((BPC, NC2, 2, 128, w_), F8_NP)
            hi = min(P, lo + w_)
            seg[:, :, :, :, :hi - lo] = blkx[:, :, :, :, lo:hi]
            im[f"xs{g}"] = np.ascontiguousarray(
                seg.transpose(3, 1, 2, 0, 4))
            im[f"wf{g}"] = wgrps[dh][g]
        in_maps.append(im)
    res = run_bass_kernel_spmd(nc, in_maps, core_ids=list(range(NCORES)))

    # ---- assemble: add D-halves, bias, elu(y)+1 ----
    y = np.empty((B, N), np.float32)
    for bg in range(NBG):
        z = res.results[NDH * bg]["z"] + res.results[NDH * bg + 1]["z"]
        for i, (s, e) in enumerate(blocks):
            idx = order[s:e]
            m = e - s
            y[BPC * bg:BPC * (bg + 1), idx] = z[0:m, BPC * i:BPC * (i + 1)].T
    y += b
    return np.where(y > 0, y + np.float32(1.0),
                    np.exp(np.minimum(y, np.float32(0.0)))).astype(np.float32)


# revision 6
# speedup vs baseline: 1.0892x; 1.0892x over previous
"""PoissonGaussianReadout forward on 8 trn2 NeuronCores.

Math (eval mode): each neuron n samples feat[b] (a [36,36,1024] image per
batch, 1024 = C*T channels) bilinearly at a fixed point mu[n], then takes a
per-neuron dot with W[n,:], adds b[n], applies elu(y)+1.

Strategy:
  - Hybrid shard 4x2: 8 cores = 4 batch-groups (4 batches each) x 2 halves
    of the contraction dim D (512 channels each).  Cores emit LINEAR
    partial sums; the host adds the halves, bias, and elu on [16,4096].
  - fp8(e4m3) x and W with DoubleRow matmuls; per-neuron dequant folded
    into a bf16 mask.
  - Sort neurons by bilinear base cell p00; 32 blocks of 128 neurons,
    each spanning a window of <=93 flat positions.  Two DoubleRow
    matmuls per block: psum[n, (b,j)] += Wblk^T @ feat-window.
  - The bilinear mask-reduce is split across engine pipelines so it is no
    longer a single-engine critical path:
      P0 blocks: DVE scalar_tensor_tensor straight from PSUM (one per
        batch, fused mult+accum).
      P2 blocks: Act copies psum -> SBUF fp32 (frees the psum bank
        early), GpSimd multiplies by the bf16 mask (stride-0 broadcast
        over the 4 batches), DVE tensor_reduce(axis=X) emits all 4 z
        values in one op.
    DVE and GpSimd run concurrently; the mix ratio balances them.
  - DMA: feat cut into 4 block-group segments; every large transfer is
    issued as TWO half-DMAs back-to-back because a single HWDGE queue
    entry only sustains ~0.2 MB/us while two concurrent entries reach
    ~0.4 MB/us.  Everything need-ordered and ungated on the sync queue:
    [W0, M0, F0, W1, F1, W2, F2, W3, F3]; the remaining mask groups ride
    the (slow-starting) scalar-engine queue; z stores issued last.
"""
import sys
sys.path.insert(0, "/opt/trn_rl_repo")

import numpy as np

from concourse import bass, mybir, tile
from concourse.bass_utils import run_bass_kernel_spmd
import bass_rust

# problem constants
B, C, T, HH, WW = 16, 64, 16, 36, 36
N, D = 4096, C * T             # 4096 neurons, 1024 input dim
P = HH * WW                    # 1296 flat positions
NCORES = 8
NBG = 4                        # batch groups
NDH = 2                        # D halves
BPC = B // NBG                 # batches per core = 4
DH = D // NDH                  # channels per core = 512
NC2 = DH // 256                # 2 double-subtile (256-chan) passes per core
PAD = 38                       # max corner offset (37) + 1
WINMAX = 128                   # psum bank: BPC*WIN <= 512 fp32
GSIZES = (6, 9, 9, 8)          # ragged block groups: small first group so
                               # the reduce pipeline starts earlier

F32 = mybir.dt.float32
BF16 = mybir.dt.bfloat16

import ml_dtypes
F8_DT = mybir.dt.float8e4
F8_NP = ml_dtypes.float8_e4m3   # max normal 240
F8_CAP = np.float32(224.0)


def _is_p3(i, nblk):
    """Measured on HW: every offload of the mask-reduce (GpSimd TT mult
    ~775ns/blk, Act activation+accum ~660ns/batch + 279ns drain, DVE
    tensor_reduce ~470ns/blk mixed) loses to the fused DVE stt chain
    (~166ns/batch).  So ALL blocks stay on the stt path."""
    return False


def _split_waits(nc, max_waits=1):
    """Walrus in this image allows only ONE sem wait per instruction.
    Hoist extra waits onto injected same-engine NoOps placed immediately
    before the owning instruction (same engine + program order => same
    semantics)."""
    k = 0
    for fn in nc.m.functions:
        for blk in fn.blocks:
            insts = blk.instructions
            out = []
            for inst in insts:
                si = inst.sync_info
                if si is not None and si.on_wait and len(si.on_wait) > max_waits:
                    waits = list(si.on_wait)
                    for w in waits[:-max_waits]:
                        nop = mybir.InstNoOp(name=f"I-wsplit-{k}", ins=[], outs=[])
                        k += 1
                        nop.engine = inst.engine
                        nop.sync_info = bass_rust.SyncInfo(
                            on_wait=[w], on_update=[]
                        )
                        out.append(nop)
                    si.on_wait = waits[-max_waits:]
                    inst.sync_info = si
                out.append(inst)
            if len(out) != len(insts):
                insts.clear()
                insts.extend(out)


def _bilinear_tables(mu):
    """Per-neuron base cell p00, corner offsets (4) in {0,1,36,37}, corner
    weights (4), replicating reference float32 arithmetic exactly."""
    one, half = np.float32(1.0), np.float32(0.5)
    g = np.clip(mu.astype(np.float32), -one, one)
    ix = (g[:, 0] + one) * np.float32(WW * 0.5) - half
    iy = (g[:, 1] + one) * np.float32(HH * 0.5) - half
    x0 = np.floor(ix)
    y0 = np.floor(iy)
    wx1 = ix - x0
    wx0 = one - wx1
    wy1 = iy - y0
    wy0 = one - wy1

    xs = [x0, x0 + one]
    ys = [y0, y0 + one]
    wxs = [wx0, wx1]
    wys = [wy0, wy1]

    x0c = np.clip(x0, 0, WW - 1).astype(np.int64)
    y0c = np.clip(y0, 0, HH - 1).astype(np.int64)
    p00 = y0c * WW + x0c

    offs = np.zeros((4, N), np.int64)
    wgts = np.zeros((4, N), np.float32)
    k = 0
    for a in range(2):          # y corner
        for bb in range(2):     # x corner
            xx, yy = xs[bb], ys[a]
            valid = (xx >= 0) & (xx <= WW - 1) & (yy >= 0) & (yy <= HH - 1)
            xi = np.clip(xx, 0, WW - 1).astype(np.int64)
            yi = np.clip(yy, 0, HH - 1).astype(np.int64)
            offs[k] = yi * WW + xi - p00
            wgts[k] = (wys[a] * wxs[bb]) * valid.astype(np.float32)
            k += 1
    assert offs.min() >= 0 and offs.max() <= 37
    return p00, offs, wgts


def _make_blocks(p00_sorted):
    """Greedy blocks of <=128 sorted neurons with window <= WINMAX."""
    blocks = []  # (start, end) into sorted order
    s = 0
    n = len(p00_sorted)
    while s < n:
        pfirst = p00_sorted[s]
        e = s
        while e < n and e - s < 128 and (p00_sorted[e] - pfirst) + PAD <= WINMAX:
            e += 1
        blocks.append((s, e))
        s = e
    return blocks


def kernel(x, mu, sigma, W, b):
    x = np.ascontiguousarray(x, dtype=np.float32)
    W = np.ascontiguousarray(W, dtype=np.float32)
    b = np.asarray(b, dtype=np.float32)

    p00, offs, wgts = _bilinear_tables(mu)
    order = np.argsort(p00, kind="stable")
    p00s = p00[order]
    blocks = _make_blocks(p00s)
    nblk = len(blocks)
    gbounds = [0]
    for gs in GSIZES:
        gbounds.append(min(gbounds[-1] + gs, nblk))
    while gbounds[-1] < nblk:
        gbounds.append(min(gbounds[-1] + GSIZES[-1], nblk))
    gbounds = sorted(set(gbounds))
    ngrp = len(gbounds) - 1

    # ---- fp8 quantization: global x scale, per-neuron W scale ----
    sx = F8_CAP / np.float32(max(np.abs(x).max(), 1e-30))
    sw = F8_CAP / np.maximum(np.abs(W).max(axis=1), 1e-30).astype(np.float32)
    Wq = (W * sw[:, None]).astype(F8_NP)    # [N, D]
    dequant = 1.0 / (sw * sx)               # [N] folded into the mask

    # per-block host data
    wins, pfirsts, ms, sblk = [], [], [], []
    mparts = []
    for i, (s, e) in enumerate(blocks):
        idx = order[s:e]
        m = e - s
        pfirst = int(p00s[s])
        win = int(p00s[e - 1]) - pfirst + PAD
        ms.append(m)
        pfirsts.append(pfirst)
        wins.append(win)
        sblk.append(s)
        # mask [128, win], fp8 dequant folded in
        mk = np.zeros((128, win), np.float32)
        rel = (p00[idx] - pfirst)  # [m]
        for k in range(4):
            np.add.at(mk[:m], (np.arange(m), rel + offs[k][idx]),
                      wgts[k][idx] * dequant[idx])
        mparts.append(mk)
    sblk.append(N)

    mask_all = np.ascontiguousarray(
        np.concatenate(mparts, axis=1)).astype(ml_dtypes.bfloat16)
    moffs = np.cumsum([0] + [w for w in wins])
    mtot = int(mask_all.shape[1])

    # feat segments: one per block group; window-union of its blocks
    seg_lo, seg_w = [], []
    for g in range(ngrp):
        lo = pfirsts[gbounds[g]]
        hi = max(pfirsts[i] + wins[i] for i in range(gbounds[g], gbounds[g + 1]))
        seg_lo.append(lo)
        seg_w.append(hi - lo)

    # W packed per group with one contiguous row per partition:
    # group layout [128, NC2, 2, sum_m(group)].
    Ws = Wq[order]                          # [N, D] sorted
    gms = [sblk[gbounds[g + 1]] - sblk[gbounds[g]] for g in range(ngrp)]
    wgrps = []                              # [dh][g] -> [128, NC2, 2, gm]
    for dh in range(NDH):
        wl = (Ws[:, dh * DH:(dh + 1) * DH].T        # [512, N]
              .reshape(NC2, 2, 128, N).transpose(2, 0, 1, 3))  # [128,NC2,2,N]
        parts = []
        for g in range(ngrp):
            lo, hi = sblk[gbounds[g]], sblk[gbounds[g + 1]]
            parts.append(np.ascontiguousarray(wl[:, :, :, lo:hi]))
        wgrps.append(parts)

    # ---- build the Bass program (same for all cores) ----
    nc = bass.Bass()
    xs_hs = [nc.declare_dram_parameter(f"xs{s}", [128, NC2, 2, BPC, seg_w[s]],
                                       F8_DT, isOutput=False)
             for s in range(ngrp)]
    wf_hs = [nc.declare_dram_parameter(f"wf{g}", [128, NC2, 2, gms[g]],
                                       F8_DT, isOutput=False)
             for g in range(ngrp)]
    mf_h = nc.declare_dram_parameter("mf", [128, mtot], BF16, isOutput=False)
    z_h = nc.declare_dram_parameter("z", [128, BPC * nblk], F32, isOutput=True)

    ADD = mybir.AluOpType.add
    MULT = mybir.AluOpType.mult
    AXX = mybir.AxisListType.X
    DR = mybir.MatmulPerfMode.DoubleRow

    with tile.TileContext(nc) as tc:
        with (
            tc.tile_pool(name="feat", bufs=1) as featp,
            tc.tile_pool(name="wpool", bufs=1) as wpool,
            tc.tile_pool(name="mpool", bufs=1) as mpool,
            tc.tile_pool(name="spool", bufs=4) as spool,
            tc.tile_pool(name="stg", bufs=4) as stgp,
            tc.tile_pool(name="mk", bufs=4) as mkp,
            tc.tile_pool(name="zpool", bufs=1) as zpool,
            tc.tile_pool(name="psum", bufs=1, space="PSUM") as psump,
        ):
            fts = [featp.tile([128, NC2, 2, BPC, seg_w[s]], F8_DT,
                              name=f"feat{s}") for s in range(ngrp)]
            mask_t = mpool.tile([128, mtot], BF16)
            zAll = zpool.tile([128, BPC * nblk], F32)
            wgs = [wpool.tile([128, NC2, 2, gms[g]], F8_DT, name=f"wg{g}")
                   for g in range(ngrp)]

            # ONE need-ordered, ungated stream on the sync queue.  The two
            # HWDGE queues share a single ~0.4 MB/us pool, so a second queue
            # only steals bandwidth from the critical first arrivals.  W and
            # feat are split in NC2 halves to keep two entries in flight (a
            # lone entry sustains only ~0.2 MB/us); the small per-group mask
            # follows its group's feat.
            for g in range(ngrp):
                for c in range(NC2):
                    nc.sync.dma_start(wgs[g][:, c], wf_hs[g][:, c])
                for c in range(NC2):
                    nc.sync.dma_start(fts[g][:, c], xs_hs[g][:, c])
                mlo = int(moffs[gbounds[g]])
                mhi = int(moffs[gbounds[g + 1]])
                nc.sync.dma_start(mask_t[:, mlo:mhi], mf_h[:, mlo:mhi])

            for g in range(ngrp):
                blks = list(range(gbounds[g], gbounds[g + 1]))
                wg = wgs[g]
                glo = sblk[gbounds[g]]
                for i in blks:
                    m, win, pfirst = ms[i], wins[i], pfirsts[i]
                    o = sblk[i] - glo
                    off = pfirst - seg_lo[g]
                    pm = psump.tile([128, BPC, win], F32,
                                    name=f"pm{i}", tag=f"pm{i % 8}")
                    for c in range(NC2):
                        nc.tensor.matmul(
                            pm[0:m, :, :],
                            wg[:, c, :, o:o + m],
                            fts[g][:, c, :, :, off:off + win],
                            start=(c == 0),
                            stop=(c == NC2 - 1),
                            perf_mode=DR,
                        )
                    mo = int(moffs[i])
                    if not _is_p3(i, nblk):
                        # DVE: fused mask-mult + accum straight out of PSUM
                        for bb in range(BPC):
                            sc = spool.tile([128, WINMAX], F32, tag=f"sv{bb}")
                            nc.vector.scalar_tensor_tensor(
                                sc[0:m, 0:win],
                                pm[0:m, bb, :],
                                0.0,
                                mask_t[0:m, mo:mo + win],
                                ADD,
                                MULT,
                                accum_out=zAll[0:m,
                                               BPC * i + bb:BPC * i + bb + 1],
                            )
                    else:
                        # Act: psum -> SBUF fp32 (frees the bank early)
                        stg = stgp.tile([128, BPC, WINMAX], F32,
                                        tag=f"st{i % 4}")
                        nc.scalar.copy(stg[0:m, :, 0:win], pm[0:m, :, :])
                        # GpSimd: multiply by mask broadcast over batches
                        base = mask_t[0:m, mo:mo + win]
                        bap = [list(p) for p in base.ap]
                        bc = bass.AP(base.tensor, base.offset,
                                     [bap[0], [0, BPC], bap[1]])
                        mk = mkp.tile([128, BPC, WINMAX], F32,
                                      tag=f"mk{i % 4}")
                        nc.gpsimd.tensor_tensor(
                            mk[0:m, :, 0:win], stg[0:m, :, 0:win], bc, MULT)
                        # Act: per-batch accumulate (keeps DVE stt-only)
                        for bb in range(BPC):
                            sa = spool.tile([128, WINMAX], F32, tag=f"sa{bb}")
                            nc.scalar.activation(
                                sa[0:m, 0:win],
                                mk[0:m, bb, 0:win],
                                mybir.ActivationFunctionType.Copy,
                                accum_out=zAll[0:m,
                                               BPC * i + bb:BPC * i + bb + 1],
                            )
            # z stores: issued after all input triggers; sync idle by then
            # z stores ride the otherwise-idle scalar queue
            zcuts = sorted(set((0, gbounds[min(3, ngrp)], nblk - 4, nblk)))
            for a, bnd in zip(zcuts[:-1], zcuts[1:]):
                sl = slice(BPC * a, BPC * bnd)
                nc.scalar.dma_start(z_h[:, sl], zAll[:, sl])

    _split_waits(nc)

    # ---- run on 8 cores: core id = bg*2 + dh ----
    xq = (x.reshape(B, D // 128, 128, P) * sx).astype(F8_NP)
    in_maps = []
    for core in range(NCORES):
        bg, dh = core // NDH, core % NDH
        blkx = xq[BPC * bg:BPC * (bg + 1),
                  4 * dh:4 * (dh + 1)].reshape(BPC, NC2, 2, 128, P)
        im = {"mf": mask_all}
        for g in range(ngrp):
            lo, w_ = seg_lo[g], seg_w[g]
            seg = np.zeros((BPC, NC2, 2, 128, w_), F8_NP)
            hi = min(P, lo + w_)
            seg[:, :, :, :, :hi - lo] = blkx[:, :, :, :, lo:hi]
            im[f"xs{g}"] = np.ascontiguousarray(
                seg.transpose(3, 1, 2, 0, 4))
            im[f"wf{g}"] = wgrps[dh][g]
        in_maps.append(im)
    res = run_bass_kernel_spmd(nc, in_maps, core_ids=list(range(NCORES)))

    # ---- assemble: add D-halves, bias, elu(y)+1 ----
    y = np.empty((B, N), np.float32)
    for bg in range(NBG):
        z = res.results[NDH * bg]["z"] + res.results[NDH * bg + 1]["z"]
        for i, (s, e) in enumerate(blocks):
            idx = order[s:e]
            m = e - s
            y[BPC * bg:BPC * (bg + 1), idx] = z[0:m, BPC * i:BPC * (i + 1)].T
    y += b
    return np.where(y > 0, y + np.float32(1.0),
                    np.exp(np.minimum(y, np.float32(0.0)))).astype(np.float32)
